# revision 3
# baseline (speedup 1.0000x reference)
"""Trainium2 Bass kernel for nn_Cell_61856118996994 (GNN message passing), v2.

Strategy
--------
Nodes are block-interleaved across the 8 cores: block b (128 nodes) lives
on core b%8 as local window k=b//8 (49 windows, 6272 rows/core).  Each
state is published via AllGather in a [core, partition, window, feat]
layout that doubles as the SBUF gather table: tab[p, c*49+k, f] =
state[(k*8+c)*128 + p, f], 12.25MB bf16, SBUF-resident.

Per spmm term, each core processes the ~100k edges whose destination
block lives on it:
  - SBUF-source transposed dma_gather (4 SWDGE queues, 896 idx/call)
    pulls source rows into [feat, edge] tiles at fabric speed,
  - PE transposes each 128-edge column tile back to [edge, feat] (PSUM),
  - ACT copies PSUM->SBUF applying the per-edge val*weight as the
    per-partition activation scale,
  - a *binary* one-hot (single DVE is_equal vs iota) per call,
  - PE scatter-matmul per destination window accumulates in PSUM,
    flushed once per (window, term) into an f32 SBUF accumulator.

Edges are streamed bank-major (int16 gather idx limit splits the table
into 2 banks) but sorted window-minor within a bank, with gather calls
issued in window-consumption order so PSUM windows stay short-lived.
Padding edges carry slot=-1 (one-hot kills them) and idx=0 (valid data).
"""
import sys

sys.path.insert(0, "/opt/trn_rl_repo")

import numpy as np

# ---------------------------------------------------------------- constants
N_NODES = 50000
N_ADJ = 6
DP = 256          # prev hidden
D = 128           # hidden
NC = 8            # cores
NBLK_T = 392      # total blocks incl pad (8*49)
KPC = 49          # windows (blocks) per core
RPC = KPC * 128   # 6272 rows per core
R = 128           # PSUM window rows
QUANT = 128       # group quantum: full-column pieces only (mixed-half
                  # 64-row piece bases inside a PSUM group crash the PE when
                  # transpose matmuls share the instruction stream)
CALL = 896        # idxs per dma_gather (HW transpose-path limit < 1024)
BANKTOK = 32768   # int16 token range per bank
NQ = 4            # SWDGE queues
SCRATCH = 16384   # dynamic dma scratch (1024-desc rings)
CSTR = [0, 2, 4]
CSTRL = [0, 2, 4, 5]


def _build_terms(idxes_seq0, idxes_seq1, idxes_res0, idxes_res1,
                 ws_seq0, ws_seq1, ws_res0, ws_res1):
    """4 passes; each a list of merged (src_state, adj_k, weight)."""
    t = [[] for _ in range(4)]
    t[0] = [(0, int(idxes_seq0[0]), float(ws_seq0[0]))]
    t[1] = [(1, int(idxes_seq0[1]), float(ws_seq0[1])),
            (0, int(idxes_res0[0]), float(ws_res0[0]))]
    t[2] = [(2, int(idxes_seq0[2]), float(ws_seq0[2])),
            (0, int(idxes_res0[1]), float(ws_res0[1])),
            (1, int(idxes_res0[2]), float(ws_res0[2]))]
    t[3] = [(3, CSTR[int(idxes_seq1[0])], float(ws_seq1[0]))]
    t[3] += [(i, CSTRL[int(idxes_res1[i])], float(ws_res1[i])) for i in range(3)]
    merged = []
    for terms in t:
        d = {}
        for s, k, w in terms:
            d[(s, k)] = d.get((s, k), 0.0) + w
        merged.append(sorted((s, k, w) for (s, k), w in d.items()))
    return merged


class AdjSched:
    __slots__ = ("E", "NT", "bank_len", "calls", "windows", "idx128", "slot8",
                 "val32")


def _build_adj(rows, cols):
    """Uniform (SPMD-shared) schedule + per-core streams for one adjacency.

    Streams are bank-major, window-minor: for bank in (0,1): for w in
    0..48: that (w,bank) group (padded to QUANT, uniform over cores).
    """
    blk = rows >> 7
    core = blk & 7
    w = blk >> 3
    slot = (rows & 127).astype(np.int64)
    sblk = cols >> 7
    rank = (sblk & 7) * KPC + (sblk >> 3)
    tok = rank * 128 + (cols & 127)
    bank = (tok >= BANKTOK).astype(np.int64)
    tok_rel = (tok - bank * BANKTOK).astype(np.int64)
    key = bank * KPC + w  # 0..97

    ngrp = 2 * KPC
    cnts = np.zeros((NC, ngrp), np.int64)
    per_core = []
    for c in range(NC):
        sel = np.flatnonzero(core == c)
        ks = key[sel]
        o = np.argsort(ks, kind="stable")
        sel = sel[o]
        cnts[c] = np.bincount(ks[o], minlength=ngrp)
        per_core.append((ks[o], tok_rel[sel], slot[sel]))

    L = ((cnts.max(axis=0) + QUANT - 1) // QUANT) * QUANT  # [98]
    Lb = L.reshape(2, KPC)
    blen = Lb.sum(axis=1)
    bpad = (-blen) % 128
    bank_len = (blen + bpad).astype(np.int64)       # [2], each %128==0
    # group offsets within the concatenated stream
    goff = np.zeros(ngrp, np.int64)
    off = 0
    bank_start = []
    for b in range(2):
        bank_start.append(off)
        for wi in range(KPC):
            goff[b * KPC + wi] = off
            off += Lb[b, wi]
        off += bpad[b]
    E = off
    NT = E // 128

    idx16 = np.zeros((NC, E), np.int16)
    slot_a = np.full((NC, E), -1, np.int8)
    pos_a = np.zeros((NC, E), np.int64)  # original edge position (for val)
    pos_a[:] = -1
    for c in range(NC):
        ks, tr, sl = per_core[c]
        if len(ks) == 0:
            continue
        run_start = np.flatnonzero(np.diff(ks, prepend=-1))
        run_lens = np.diff(np.append(run_start, len(ks)))
        rnk = np.arange(len(ks)) - np.repeat(run_start, run_lens)
        dest = goff[ks] + rnk
        idx16[c, dest] = tr.astype(np.int16)
        slot_a[c, dest] = sl.astype(np.int8)
    # recover original positions for val lookup
    for c in range(NC):
        sel = np.flatnonzero(core == c)
        ks = key[sel]
        o = np.argsort(ks, kind="stable")
        sel = sel[o]
        ks = ks[o]
        run_start = np.flatnonzero(np.diff(ks, prepend=-1))
        run_lens = np.diff(np.append(run_start, len(ks)))
        rnk = np.arange(len(ks)) - np.repeat(run_start, run_lens)
        dest = goff[ks] + rnk
        pos_a[c, dest] = sel

    # idx layout: position j -> partition j%16, col j//16; replicate to 128
    idxw = idx16.reshape(NC, E // 16, 16).transpose(0, 2, 1)  # [NC,16,E/16]
    idx128 = np.ascontiguousarray(np.tile(idxw, (1, 8, 1)))   # [NC,128,E/16]
    slot8 = np.ascontiguousarray(
        slot_a.reshape(NC, NT, 128).transpose(0, 2, 1))       # [NC,128,NT]

    # gather calls: per bank chopped at CALL grid; issue order by the
    # window containing the call's first edge
    calls = []
    for b in range(2):
        s0 = bank_start[b]
        e = 0
        while e < bank_len[b]:
            ee = min(e + CALL, int(bank_len[b]))
            # first window of this call
            gpos = s0 + e
            wfirst = int(np.searchsorted(goff[b * KPC:(b + 1) * KPC], gpos,
                                         side="right") - 1)
            calls.append((wfirst, b, e, ee))
            e = ee
    calls.sort(key=lambda t: (t[0], t[1], t[2]))

    # window pieces: per w, both banks: (global col, p0, p1)
    windows = []
    for wi in range(KPC):
        pieces = []
        for b in range(2):
            gs = int(goff[b * KPC + wi])
            ge = gs + int(Lb[b, wi])
            e = gs
            while e < ge:
                col = e // 128
                p0 = e - col * 128
                p1 = min(ge - col * 128, 128)
                pieces.append((col, p0, p1))
                e = col * 128 + p1
        windows.append(pieces)

    ps = AdjSched()
    ps.E, ps.NT, ps.bank_len = E, NT, (int(bank_len[0]), int(bank_len[1]))
    ps.calls, ps.windows = calls, windows
    ps.idx128, ps.slot8 = idx128, slot8
    ps.val32 = pos_a  # placeholder: positions; vals filled in _prepare
    return ps


def _val_stream(ps, vals, wgt):
    """Per-term [NC, 128, NT] f32 val stream (weight folded in)."""
    v = np.zeros((NC, ps.E), np.float32)
    pos = ps.val32
    m = pos >= 0
    v[m] = vals[pos[m]] * wgt
    return np.ascontiguousarray(v.reshape(NC, ps.NT, 128).transpose(0, 2, 1))


def _build_program(scheds, terms):
    import concourse.bass as bass
    import concourse.tile as tile
    from concourse import bacc, mybir

    f32 = mybir.dt.float32
    bf16 = mybir.dt.bfloat16
    i8 = mybir.dt.int8
    i16 = mybir.dt.int16
    nc = bacc.Bacc("TRN2", target_bir_lowering=False, debug=False,
                   enable_asserts=False, num_devices=NC,
                   dynamic_dma_scratch_size=SCRATCH, num_swdge_queues=NQ)

    xT_d = nc.dram_tensor("xT", [DP, RPC], bf16, kind="ExternalInput").ap()
    W_d = nc.dram_tensor("W", [DP, D], bf16, kind="ExternalInput").ap()
    b_d = nc.dram_tensor("bias", [1, D], bf16, kind="ExternalInput").ap()
    ones_d = nc.dram_tensor("ones", [1, 128], bf16, kind="ExternalInput").ap()
    iota_d = nc.dram_tensor("iota", [128, R], i8, kind="ExternalInput").ap()
    ident_d = nc.dram_tensor("ident", [128, 128], bf16,
                             kind="ExternalInput").ap()
    adj_keys = sorted(scheds)
    idx_d, slot_d = {}, {}
    for k in adj_keys:
        ps = scheds[k]
        idx_d[k] = nc.dram_tensor(f"idx{k}", [128, ps.E // 16], i16,
                                  kind="ExternalInput").ap()
        slot_d[k] = nc.dram_tensor(f"slot{k}", [128, ps.NT], i8,
                                   kind="ExternalInput").ap()
    val_d = {}
    for q in range(4):
        for ti, (s, k, w) in enumerate(terms[q]):
            ps = scheds[k]
            val_d[(q, ti)] = nc.dram_tensor(
                f"val{q}_{ti}", [128, ps.NT], f32, kind="ExternalInput").ap()
    out_d = nc.dram_tensor("out", [RPC, D], bf16, kind="ExternalOutput").ap()

    with tile.TileContext(nc) as tc:
        with tc.tile_pool(name="persist", bufs=1) as pp, \
             tc.tile_pool(name="dram", bufs=1, space="DRAM") as dram:
            iota_s = pp.tile([128, R], i8)
            nc.sync.dma_start(iota_s[:], iota_d[:])
            ident_s = pp.tile([128, 128], bf16)
            nc.sync.dma_start(ident_s[:], ident_d[:])
            tab = pp.tile([128, NBLK_T, D], bf16)
            acc = pp.tile([128, KPC, D], f32)
            acc16 = pp.tile([128, KPC, D], bf16)

            dbg = globals().get("DEBUG_STAGE", None)
            reps = globals().get("TIMING_REPS", 1)
            for _rep in range(reps):
                _emit_once(nc, tc, bass, dram, scheds, terms, dbg, _rep,
                           iota_s, ident_s, tab, acc, acc16,
                           xT_d, W_d, ones_d, b_d, idx_d, slot_d, val_d,
                           out_d)
    return nc


def _emit_once(nc, tc, bass, dram, scheds, terms, dbg, rep,
               iota_s, ident_s, tab, acc, acc16,
               xT_d, W_d, ones_d, b_d, idx_d, slot_d, val_d, out_d):
    from concourse import mybir
    f32 = mybir.dt.float32
    bf16 = mybir.dt.bfloat16
    i8 = mybir.dt.int8
    i16 = mybir.dt.int16

    bounces = [dram.tile([128, RPC], bf16, name=f"bounce{t}_r{rep}")
               for t in range(4)]
    ags = [dram.tile([NC * 128, RPC], bf16, addr_space="Shared",
                     name=f"ag{t}_r{rep}") for t in range(4)]

    cur_tab = [-1]

    def load_table(p):
        nc.sync.dma_start(
            tab[:].rearrange("p (c k) f -> p c (k f)", c=NC),
            ags[p][:].rearrange("(c p) x -> p c x", p=128))
        cur_tab[0] = p

    def publish(p):
        # acc16 [128, KPC, D] -> bounce [128, KPC*D] (contig per partition)
        nc.sync.dma_start(bounces[p][:],
                          acc16[:].rearrange("p k f -> p (k f)"))
        if globals().get("NO_AG", False):
            nc.sync.dma_start(
                ags[p][:].rearrange("(c p) f -> c p f", c=NC),
                bounces[p][:].unsqueeze(0).broadcast_to([NC, 128, RPC]))
        else:
            nc.gpsimd.collective_compute(
                "AllGather", bass.mybir.AluOpType.bypass,
                replica_groups=[list(range(NC))],
                ins=[bounces[p][:].opt()], outs=[ags[p][:].opt()])
        # table preload: the next pass's first term sources this state
        load_table(p)

    def dump_acc16():
        nc.sync.dma_start(
            out_d[:].rearrange("(k p) f -> p k f", p=128),
            acc16[:])

    # ---------------- pass 0: h0 = x @ W + b ----------------
    with tc.tile_pool(name=f"p0_{rep}", bufs=1) as p0, \
         tc.tile_pool(name=f"ps0_{rep}", bufs=2, space="PSUM") as ps0:
        xT_s = p0.tile([128, 2, RPC], bf16)
        nc.sync.dma_start(xT_s[:], xT_d.rearrange("(c k) r -> k c r", k=128))
        W_s = p0.tile([128, 2, D], bf16)
        nc.sync.dma_start(W_s[:], W_d.rearrange("(c k) n -> k c n", k=128))
        ones_s = p0.tile([1, 128], bf16)
        nc.sync.dma_start(ones_s[:], ones_d[:])
        b_s = p0.tile([1, D], bf16)
        nc.sync.dma_start(b_s[:], b_d[:])
        for t in range(KPC):
            pw = ps0.tile([128, D], f32)
            for c in range(2):
                nc.tensor.matmul(
                    pw[:], xT_s[:, c, t * 128:(t + 1) * 128],
                    W_s[:, c, :], start=(c == 0), stop=False)
            nc.tensor.matmul(pw[:], ones_s[:], b_s[:], start=False, stop=True)
            nc.scalar.copy(acc16[:, t, :], pw[:])
    if dbg == 0:
        dump_acc16()
        return
    publish(0)
    if dbg == 0.5:
        dump_acc16()
        return

    # ------------- spmm passes -------------
    for q in range(4 if dbg is None else dbg):
        with tc.tile_pool(name=f"ar{q}_{rep}", bufs=1) as arp, \
             tc.tile_pool(name=f"sv{q}_{rep}", bufs=2) as svp, \
             tc.tile_pool(name=f"g{q}_{rep}", bufs=4) as gp, \
             tc.tile_pool(name=f"oh{q}_{rep}", bufs=6) as ohp, \
             tc.tile_pool(name=f"gs{q}_{rep}", bufs=8) as gsp, \
             tc.tile_pool(name=f"pst{q}_{rep}", bufs=4, space="PSUM") as pst, \
             tc.tile_pool(name=f"psw{q}_{rep}", bufs=3, space="PSUM") as psw:
            # newest state first: it is already resident from publish()
            order = sorted(terms[q], key=lambda t: (-t[0], t[1]))
            for ti, (s_state, k, wgt) in enumerate(order):
                ps = scheds[k]
                first = (ti == 0)
                if s_state != cur_tab[0]:
                    load_table(s_state)
                arena = arp.tile([128, ps.E // 16], i16, tag="idx")
                nc.sync.dma_start(arena[:], idx_d[k][:])
                slot_t = svp.tile([128, ps.NT], i8, tag="slot")
                nc.sync.dma_start(slot_t[:], slot_d[k][:])
                val_t = svp.tile([128, ps.NT], f32, tag="val")
                ti_orig = terms[q].index((s_state, k, wgt))
                nc.sync.dma_start(val_t[:], val_d[(q, ti_orig)][:])

                bank_coloff = (0, ps.bank_len[0] // 128)
                bank_ap = (tab[:, :BANKTOK // 128, :],
                           tab[:, BANKTOK // 128:, :])

                # per-call emitted state: col -> (gst_tile, oh_tile, subcol)
                gs_tiles = {}
                callptr = [0]

                def emit_call(ci, qsel):
                    wf, b, e0, e1 = ps.calls[ci]
                    n = e1 - e0
                    gt = gp.tile([128, 1, CALL], bf16, tag="g")
                    coff = bank_coloff[b] + e0 // 128
                    a0 = (bank_coloff[b] * 128 + e0) // 16
                    if globals().get("NO_GATHER", False):
                        nc.vector.memset(gt[:, :, :n], 0.0)
                    else:
                        nc.gpsimd.dma_gather(
                            gt[:, :, :n], bank_ap[b],
                            arena[:, a0:a0 + n // 16],
                            num_idxs=n, num_idxs_reg=n, elem_size=D,
                            transpose=True, single_packet=True,
                            queue_num=qsel,
                            sbuf_tokens_per_rank=128,
                            sbuf_free_dim_per_rank=2 * D,
                            sbuf_free_dim_pad_per_rank=0, sbuf_byte_offset=0)
                    ncol = n // 128
                    oh = ohp.tile([128, CALL // 128, R], bf16, tag="oh")
                    nc.vector.tensor_tensor(
                        oh[:, :ncol, :],
                        iota_s[:].unsqueeze(1).broadcast_to([128, ncol, R]),
                        slot_t[:, coff:coff + ncol].unsqueeze(2)
                            .broadcast_to([128, ncol, R]),
                        bass.mybir.AluOpType.is_equal)
                    gst = gsp.tile([128, CALL // 128, 128], bf16, tag="gs")
                    for gi, c0 in enumerate(range(0, ncol, 4)):
                        nb = min(4, ncol - c0)
                        # transpose via normal matmul: g = gT^T @ I
                        pt = pst.tile([128, 4, 128], f32, tag="pt")
                        for j in range(nb):
                            cc = c0 + j
                            nc.tensor.matmul(pt[:, j, :],
                                             gt[:, 0, cc * 128:(cc + 1) * 128],
                                             ident_s[:], start=True, stop=True)
                        if (globals().get('DVE_COPY_EVERY', 0) and gi % globals().get('DVE_COPY_EVERY', 0) == 0):
                            # fused 4-col scale on DVE (val bcast along feat)
                            nc.vector.tensor_tensor(
                                gst[:, c0:c0 + nb, :], pt[:, :nb, :],
                                val_t[:, coff + c0:coff + c0 + nb]
                                    .unsqueeze(2).broadcast_to([128, nb, 128]),
                                bass.mybir.AluOpType.mult)
                        else:
                            for j in range(nb):
                                cc = c0 + j
                                nc.scalar.activation(
                                    gst[:, cc, :], pt[:, j, :],
                                    bass.mybir.ActivationFunctionType.Copy,
                                    scale=val_t[:, coff + cc:coff + cc + 1])
                    for cc in range(ncol):
                        gs_tiles[coff + cc] = (gst, oh, cc)

                nwin = globals().get("LIMIT_WINDOWS", KPC)
                for wi in range(nwin):
                    while (callptr[0] < len(ps.calls)
                           and ps.calls[callptr[0]][0] <= wi):
                        emit_call(callptr[0], callptr[0] % NQ)
                        callptr[0] += 1
                    pieces = ps.windows[wi]
                    if not pieces:
                        continue
                    pw = psw.tile([R, D], f32, tag="pw")
                    npc = len(pieces)
                    if not globals().get("NO_MM", False):
                        for pi, (col, p0, p1) in enumerate(pieces):
                            gst, oh, cc = gs_tiles[col]
                            nc.tensor.matmul(
                                pw[:], oh[p0:p1, cc, :], gst[p0:p1, cc, :],
                                start=(pi == 0), stop=(pi == npc - 1))
                    else:
                        nc.vector.memset(pw[:], 0.0)
                    if globals().get("NO_FLUSH", False):
                        nc.vector.memset(acc[:, wi, :], 0.0)
                    elif first:
                        nc.vector.tensor_copy(acc[:, wi, :], pw[:])
                    else:
                        nc.vector.tensor_add(acc[:, wi, :], acc[:, wi, :],
                                             pw[:])
        if dbg == q + 1:
            nc.scalar.copy(acc16[:], acc[:])
            dump_acc16()
            return
        if q < 3:
            nc.scalar.copy(acc16[:], acc[:])
            publish(q + 1)

    # ---------------- LayerNorm + GELU (chunked over windows) ----------------
    with tc.tile_pool(name=f"lnc_{rep}", bufs=1) as lc, \
         tc.tile_pool(name=f"ln_{rep}", bufs=3) as lp:
        eps_t = lc.tile([128, 1], f32)
        nc.vector.memset(eps_t[:], 1e-5)
        zero_t = lc.tile([128, 1], f32)
        nc.vector.memset(zero_t[:], 0.0)
        CH = 7
        for w0 in range(0, KPC, CH):
            nw = min(CH, KPC - w0)
            a = acc[:, w0:w0 + nw, :]
            ms = lp.tile([128, CH, 1], f32, tag="ms")
            nc.vector.reduce_sum(ms[:, :nw, :], a,
                                 axis=bass.mybir.AxisListType.X)
            mu_t = lp.tile([128, CH, 1], f32, tag="mu")
            nc.scalar.mul(mu_t[:, :nw, :], ms[:, :nw, :], 1.0 / D)
            xm = lp.tile([128, CH, D], f32, tag="xm")
            nc.vector.tensor_tensor(
                xm[:, :nw, :], a,
                mu_t[:, :nw, :].broadcast_to([128, nw, D]),
                bass.mybir.AluOpType.subtract)
            sq = lp.tile([128, CH, D], f32, tag="sq")
            nc.scalar.square(sq[:, :nw, :], xm[:, :nw, :])
            vs = lp.tile([128, CH, 1], f32, tag="vs")
            nc.vector.reduce_sum(vs[:, :nw, :], sq[:, :nw, :],
                                 axis=bass.mybir.AxisListType.X)
            std = lp.tile([128, CH, 1], f32, tag="std")
            nc.scalar.activation(
                std[:, :nw, :], vs[:, :nw, :],
                bass.mybir.ActivationFunctionType.Sqrt,
                bias=eps_t[:], scale=1.0 / D)
            rinv = lp.tile([128, CH, 1], f32, tag="ri")
            nc.vector.reciprocal(rinv[:, :nw, :], std[:, :nw, :])
            normed = lp.tile([128, CH, D], f32, tag="nm")
            nc.vector.tensor_tensor(
                normed[:, :nw, :], xm[:, :nw, :],
                rinv[:, :nw, :].broadcast_to([128, nw, D]),
                bass.mybir.AluOpType.mult)
            gl = lp.tile([128, CH, D], bf16, tag="gl")
            nc.scalar.activation(
                gl[:, :nw, :], normed[:, :nw, :],
                bass.mybir.ActivationFunctionType.Gelu,
                bias=zero_t[:])
            nc.sync.dma_start(
                out_d[w0 * 128:(w0 + nw) * 128, :]
                    .rearrange("(k p) f -> p k f", p=128),
                gl[:, :nw, :])


def _prepare(inputs):
    from concourse import mybir
    bf16 = mybir.dt.np(mybir.dt.bfloat16)

    x = np.asarray(inputs["x"], np.float32)
    adj_rows = np.asarray(inputs["adj_rows"])
    adj_cols = np.asarray(inputs["adj_cols"])
    adj_vals = np.asarray(inputs["adj_vals"], np.float32)
    W = np.asarray(inputs["W"], np.float32)
    b = np.asarray(inputs["b"], np.float32)

    terms = _build_terms(
        np.asarray(inputs["idxes_seq0"]), np.asarray(inputs["idxes_seq1"]),
        np.asarray(inputs["idxes_res0"]), np.asarray(inputs["idxes_res1"]),
        np.asarray(inputs["ws_seq0"]), np.asarray(inputs["ws_seq1"]),
        np.asarray(inputs["ws_res0"]), np.asarray(inputs["ws_res1"]))
    used = sorted({k for t in terms for (s, k, w) in t})
    scheds = {k: _build_adj(adj_rows[k], adj_cols[k]) for k in used}
    globals()["_last_scheds"] = scheds
    globals()["_last_terms"] = terms
    nc = _build_program(scheds, terms)
    nc.compile()
    from concourse.bass_interp import get_hw_module
    nc.m = get_hw_module(nc.m)

    iota = np.broadcast_to(np.arange(R, dtype=np.int8), (128, R)).copy()
    ones = np.ones((1, 128), bf16)
    ident = np.eye(128, dtype=np.float32).astype(bf16)
    b_row = b.reshape(1, D).astype(bf16)

    # x sharding: block-interleaved; pad to 50176 rows
    xp = np.zeros((NBLK_T * 128, DP), np.float32)
    xp[:N_NODES] = x
    xb = xp.reshape(NBLK_T, 128, DP)

    val_streams = {}
    for q in range(4):
        for ti, (s, k, wgt) in enumerate(terms[q]):
            val_streams[(q, ti)] = _val_stream(scheds[k], adj_vals[k], wgt)

    in_maps = []
    for c in range(NC):
        xs = xb[c::NC].reshape(RPC, DP)     # blocks c, c+8, ... (49 blocks)
        m = {"xT": np.ascontiguousarray(xs.T).astype(bf16),
             "W": W.astype(bf16), "bias": b_row, "ones": ones,
             "iota": iota, "ident": ident}
        for k, ps in scheds.items():
            m[f"idx{k}"] = ps.idx128[c]
            m[f"slot{k}"] = ps.slot8[c]
        for (q, ti), v in val_streams.items():
            m[f"val{q}_{ti}"] = v[c]
        in_maps.append(m)
    return nc, in_maps


def make_runner(nc, in_maps):
    """AOT-compiled SPMD runner (same contract as baseline kernel)."""
    import jax
    import jax.numpy as jnp
    from jax.sharding import Mesh, PartitionSpec, NamedSharding
    try:
        from jax import shard_map
    except ImportError:
        from jax.experimental.shard_map import shard_map
    from concourse import mybir
    from concourse.bass2jax import (
        _bass_exec_p, partition_id_tensor, install_neuronx_cc_hook)
    import time

    install_neuronx_cc_hook()
    partition_name = (nc.partition_id_tensor.name
                      if nc.partition_id_tensor else None)
    in_names, out_names, out_avals = [], [], []
    for alloc in nc.m.functions[0].allocations:
        if not isinstance(alloc, mybir.MemoryLocationSet):
            continue
        name = alloc.memorylocations[0].name
        if alloc.kind == "ExternalInput":
            if name != partition_name:
                in_names.append(name)
        elif alloc.kind == "ExternalOutput":
            out_names.append(name)
            out_avals.append(jax.core.ShapedArray(
                tuple(alloc.tensor_shape), mybir.dt.np(alloc.dtype)))
    n_params = len(in_names)
    n_outs = len(out_avals)
    all_in_names = in_names + out_names + (
        [partition_name] if partition_name else [])
    donate = tuple(range(n_params, n_params + n_outs))

    def _exec_once(*operands):
        return tuple(_bass_exec_p.bind(
            *operands, out_avals=tuple(out_avals),
            in_names=tuple(all_in_names), out_names=tuple(out_names),
            lowering_input_output_aliases=(),
            sim_require_finite=True, sim_require_nnan=True, nc=nc))

    def _body(*args):
        operands = list(args)
        if partition_name is not None:
            operands.append(partition_id_tensor())
        return _exec_once(*operands)

    devices = jax.devices()[:NC]
    mesh = Mesh(np.asarray(devices), ("core",))
    spec = PartitionSpec("core")
    smap_kwargs = dict(mesh=mesh, in_specs=(spec,) * (n_params + n_outs),
                       out_specs=(spec,) * n_outs)

    def _shard(fn):
        try:
            return shard_map(fn, check_vma=False, **smap_kwargs)
        except TypeError:
            return shard_map(fn, check_rep=False, **smap_kwargs)

    sharded = jax.jit(_shard(_body), donate_argnums=donate, keep_unused=True)
    sh = NamedSharding(mesh, spec)
    zshapes = [(NC * a.shape[0], *a.shape[1:]) for a in out_avals]
    zdtypes = [a.dtype for a in out_avals]
    zeros_fn = jax.jit(
        lambda: tuple(jnp.zeros(s, d) for s, d in zip(zshapes, zdtypes)),
        out_shardings=tuple(sh for _ in out_avals))

    state = {}

    def stage():
        t0 = time.perf_counter()
        concat = [np.concatenate(
            [np.asarray(in_maps[c][n]) for c in range(NC)], axis=0)
            for n in in_names]
        dev = jax.device_put(concat, [sh] * n_params)
        jax.block_until_ready(dev)
        state["dev_in"] = dev
        return time.perf_counter() - t0

    def run():
        t0 = time.perf_counter()
        z = zeros_fn()
        out = sharded(*state["dev_in"], *z)
        jax.block_until_ready(out)
        state["out"] = out
        return time.perf_counter() - t0

    def fetch():
        oi = out_names.index("out")
        full = np.asarray(state["out"][oi]).astype(np.float32)
        # rows come back block-interleaved: [core, k, p] -> node (k*8+c)*128+p
        full = full.reshape(NC, KPC, 128, D).transpose(1, 0, 2, 3)
        return full.reshape(NBLK_T * 128, D)[:N_NODES]

    return stage, run, fetch


def kernel(**inputs) -> np.ndarray:
    nc, in_maps = _prepare(inputs)
    stage, run, fetch = make_runner(nc, in_maps)
    stage()
    run()
    return fetch()
